# revision 1
# baseline (speedup 1.0000x reference)
"""Trainium2 Bass kernel for the S-LSTM (sentence-state LSTM) classifier.

Data-parallel over batch: 8 cores x 4 examples. Everything on-chip runs in a
"transposed" layout: feature channels on SBUF partitions, (example, position)
flattened on the free dim (4*128 = 512 columns). The per-step gate GEMM runs
in fp8e4m3 DoubleRow mode (256 K-rows per 512-cycle matmul): the h state is
stored as fp8 scaled by SH, weights are pre-scaled by SW on the host, and the
descale folds into the eviction activation's scale parameter.

Position shifts (h_{i-1}, h_{i+1}, c shifts) are materialized into dense
shadow tiles (hL/hR, cL/cR) by SBUF->SBUF DMAs issued from the Scalar HWDGE
queue; per-example boundary zero columns are written once at init. This keeps
every DoubleRow moving operand a clean 3-D [128, 2, N] view and every
elementwise operand contiguous 16-bit (DVE 2x packing).

The x-part of the gates (x @ Wg_x, constant across steps) is folded into the
same psum accumulation as 2 extra DoubleRow matmuls per chunk (the x weight
chunks ride in the same per-chunk weight DMA), so gate eviction is a single
Scalar activation(Exp/Sigmoid/Tanh, bias=bg, scale=descale) reading PSUM --
no Vector op gates PSUM recycling.

The global-node ("g") part of ctx is rank-1 along positions: gg = g @ Wg_g is
computed once per step as a tiny M=4 fp8-DoubleRow GEMM, then folded into the
big GEMM as an extra K chunk against a constant 0/1 selector matrix.
"""

import ml_dtypes
import numpy as np

import concourse.bass as bass
import concourse.mybir as mybir
from concourse import bacc
import concourse.tile as tile
from concourse.bass_utils import run_bass_kernel_spmd

F32 = mybir.dt.float32
F32R = mybir.dt.float32r
F16 = mybir.dt.float16
BF16 = mybir.dt.bfloat16
FP8 = mybir.dt.float8e4
I32 = mybir.dt.int32
AL = mybir.AluOpType
AF = mybir.ActivationFunctionType
AX = mybir.AxisListType
DR = mybir.MatmulPerfMode.DoubleRow

B, L, V, E, H, DOUT = 32, 128, 30000, 300, 512, 5
NUM_STEPS = 5
NCORES = 8
BL = B // NCORES          # 4 examples per core
N = BL * L                # 512 free columns
EP = 384                  # E padded to 3*128 (f32r h0 GEMM)
HC = H // 128             # 4 H chunks
GC = 7 * H // 128         # 28 gate output chunks
KHH = 3 * HC              # 12 K chunks for hl/h/hr
EC = EP // 128            # 3 E chunks
EP4 = 512                 # E padded to 4*128 (fp8 DoubleRow pairs)
EC4 = EP4 // 128          # 4 E chunks for the fp8 x-part
KT = KHH + EC4            # 16 K chunks in the combined per-chunk weight tile
GG_W = 7 * H + H          # 4096: [Wg_g | Wfi_g] columns
GGC = GG_W // 512         # 8
DP = 8                    # DOUT padded to even size for fp32r matmul

# fp8 scales (powers of two; descales fold into activation scale params).
# SH*SW == SX*SWX == SG*SWCAT so every contribution to a gate psum carries the
# same scale and one descale applies.
SH = 16.0                 # h state scale (|h| <= 1 -> <= 16)
SW = 2048.0               # Wg/Wfi weight scale (absmax ~0.11 -> <= 222 < 240)
SX = 16.0                 # embedding scale (absmax ~0.52 -> <= 8.4)
SWX = 2048.0              # Wg_x weight scale
SG = 16.0                 # g state scale (|g| <= 1)
SWCAT = 2048.0            # Wg_gcat/Wfi_g weight scale
DS = 1.0 / (SH * SW)      # uniform descale for the fp8 GEMM psums


def build_nc():
    nc = bacc.Bacc(trn_type="TRN2", target_bir_lowering=False)

    d = {}

    def din(name, shape, dt=F32):
        d[name] = nc.dram_tensor(name, list(shape), dt, kind="ExternalInput")
        return d[name]

    # weights are host-pre-tiled so every DMA reads contiguous HBM
    embed_d = din("embed", (V, E))
    wg_full = din("wg_full", (GC, 128, KHH, 128), FP8)  # hl/h/hr K chunks
    wg_x = din("wg_x", (4, 128, GC // 4, EC4, 128), FP8)
    wg_gcat = din("wg_gcat", (GGC, 128, HC, 512), BF16)
    wfi_h = din("wfi_h", (128, HC, H), FP8)
    wfo_d = din("wfo", (HC, 128, 2 * HC, 256), BF16)    # [Wgf | Wgo] m-pairs
    w0_d = din("w0", (HC, 128, EC, 128), F32R)
    w1_d = din("w1", (128, 2 * HC, HC, 128), BF16)
    w2_d = din("w2", (128, 2 * HC, DP), BF16)
    bg_t = din("bg_t", (128, GC))
    b0_t = din("b0_t", (128, HC))
    bfi_t = din("bfi_t", (128, HC))
    bgf_t = din("bgf_t", (128, HC))
    bgo_t = din("bgo_t", (128, HC))
    b1_t = din("b1_t", (128, 2 * HC))
    b2_r = din("b2_r", (BL, DP))
    sel_d = din("sel", (128, N), F32R)
    msel_d = din("msel", (128, N), F32R)
    pen_d = din("pen_rows", (BL, H), F32R)
    ident_d = din("ident", (128, 128))
    identb_d = din("ident_bf", (128, 128), BF16)
    tok_d = din("tok_idx", (128, BL), I32)       # column e = tokens of example e
    mask_d = din("mask_rep", (128, N), BF16)
    invlen_d = din("invlen_rep", (128, BL))

    out_d = nc.dram_tensor("out", [BL, DOUT], F32, kind="ExternalOutput")

    with tile.TileContext(nc) as tc:
        with (
            tc.tile_pool(name="psumA", bufs=4, space="PSUM") as psumA,
            tc.tile_pool(name="psumB", bufs=2, space="PSUM") as psumB,
            tc.tile_pool(name="psumT", bufs=2, space="PSUM") as psumT,
            tc.tile_pool(name="gates", bufs=10) as p_gate,
            tc.tile_pool(name="tmp", bufs=12) as p_tmp,
            tc.tile_pool(name="wg", bufs=10) as p_wg,
            tc.tile_pool(name="wcat", bufs=4) as p_wcat,
            tc.tile_pool(name="wgfgo", bufs=4) as p_wgfgo,
            tc.tile_pool(name="wx", bufs=3) as p_wx,
            tc.tile_pool(name="small", bufs=28) as p_small,
            tc.tile_pool(name="state", bufs=1) as p_state,
        ):
            # ---------------- persistent state ----------------
            def T(shape, name, dt=F32):
                return p_state.tile(shape, dt, name=name, tag=name)

            # h/c stored densely; +-1 position shifts live in shadow tiles
            # kept in sync by SBUF->SBUF DMAs (boundary zeros written once).
            hD = [T([128, HC, N], f"hD{i}", FP8) for i in range(2)]
            hL = [T([128, HC, N], f"hL{i}", FP8) for i in range(2)]
            hR = [T([128, HC, N], f"hR{i}", FP8) for i in range(2)]
            cD = [T([128, HC, N], f"cD{i}", BF16) for i in range(2)]
            cL = [T([128, HC, N], f"cL{i}", BF16) for i in range(2)]
            cR = [T([128, HC, N], f"cR{i}", BF16) for i in range(2)]
            gT = [T([128, HC, BL], f"gT{i}", BF16) for i in range(2)]
            cgT = [T([128, HC, BL], f"cgT{i}") for i in range(2)]
            xT = T([128, EC, N], "xT", F32R)
            xT8 = T([128, EC4, N], "xT8", FP8)
            gate_x = T([128, GC, N], "gate_x", BF16)
            # gg rows 0:BL hold g @ [Wg_g | Wfi_g]; rows BL:128 stay zero so the
            # selector matmul can contract over a full K=128.
            gg_sb = T([128, GG_W], "gg_sb", F32R)
            x_nat = T([128, BL, EP], "x_nat")
            idx_sb = T([128, BL], "idx_sb", I32)
            mask_sb = T([128, N], "mask_sb", BF16)
            invlen_sb = T([128, BL], "invlen_sb")
            sel_sb = T([128, N], "sel_sb", F32R)
            msel_sb = T([128, N], "msel_sb", F32R)
            ident_sb = T([128, 128], "ident_sb")
            identb_sb = T([128, 128], "identb_sb", BF16)
            wfi_sb = T([128, HC, H], "wfi_sb", FP8)
            w1_sb = T([128, 2 * HC, HC, 128], "w1_sb", BF16)
            w2_sb = T([128, 2 * HC, DP], "w2_sb", BF16)
            a1T = T([128, 2 * HC, BL], "a1T", BF16)
            bg_sb = T([128, GC], "bg_sb")
            b0_sb = T([128, HC], "b0_sb")
            bfi_sb = T([128, HC], "bfi_sb")
            bgf_sb = T([128, HC], "bgf_sb")
            bgo_sb = T([128, HC], "bgo_sb")
            b1_sb = T([128, 2 * HC], "b1_sb")
            b2_sb = T([BL, DP], "b2_sb")

            def mask3():
                return mask_sb[:].rearrange("p (e l) -> p e l", l=L)

            def v3(t):
                return t[:].rearrange("p (e l) -> p e l", l=L)

            def v3a(ap):
                return ap.rearrange("p (e l) -> p e l", l=L)

            def emit_shift_dmas(td, tl, tr, hk):
                # tl[i] = t[i-1], tr[i] = t[i+1]; per-example boundary cols
                # stay zero from the prologue memset. Issued from the Scalar
                # HWDGE queue so waits never block the Sync weight-load queue.
                d3 = v3a(td[:, hk])
                nc.scalar.dma_start(v3a(tl[:, hk])[:, :, 1:L], d3[:, :, 0 : L - 1])
                nc.scalar.dma_start(v3a(tr[:, hk])[:, :, 0 : L - 1], d3[:, :, 1:L])

            def tmp2(name, dt=BF16):
                return p_tmp.tile([128, N], dt, name=name, tag="tmp")

            def tmp3(name, dt=BF16):
                return p_tmp.tile([128, BL, L], dt, name=name, tag="tmp")

            def sm(name):
                return p_small.tile([128, BL], F32, name=name, tag="sm")

            # ---------------- prologue: loads ----------------
            nc.sync.dma_start(idx_sb[:], tok_d.ap())
            nc.sync.dma_start(mask_sb[:], mask_d.ap())
            nc.sync.dma_start(invlen_sb[:], invlen_d.ap())
            nc.sync.dma_start(sel_sb[:], sel_d.ap())
            nc.sync.dma_start(msel_sb[:], msel_d.ap())
            nc.sync.dma_start(ident_sb[:], ident_d.ap())
            nc.sync.dma_start(identb_sb[:], identb_d.ap())
            nc.sync.dma_start(wfi_sb[:], wfi_h.ap())
            nc.sync.dma_start(w1_sb[:], w1_d.ap())
            nc.sync.dma_start(w2_sb[:], w2_d.ap())
            for t_sb, t_d in (
                (bg_sb, bg_t), (b0_sb, b0_t), (bfi_sb, bfi_t),
                (bgf_sb, bgf_t), (bgo_sb, bgo_t), (b1_sb, b1_t), (b2_sb, b2_r),
            ):
                nc.sync.dma_start(t_sb[:], t_d.ap())

            # zero only what a step actually reads before writing: buffer-0
            # c tiles (c0 = 0), shadow-tile boundary columns (written once,
            # never touched again), cg0, pads. Split across engines.
            nc.gpsimd.memset(cD[0][:], 0.0)
            nc.vector.memset(cL[0][:], 0.0)
            nc.gpsimd.memset(cR[0][:], 0.0)
            for t in (hL[1], hR[1], cL[1], cR[1]):
                bv = t[:].rearrange("p q (e l) -> p (q e) l", l=L)
                nc.vector.memset(bv[:, :, 0:1], 0.0)
                nc.vector.memset(bv[:, :, L - 1 : L], 0.0)
            for t in (hL[0], hR[0]):
                bv = t[:].rearrange("p q (e l) -> p (q e) l", l=L)
                nc.vector.memset(bv[:, :, 0:1], 0.0)
                nc.vector.memset(bv[:, :, L - 1 : L], 0.0)
            nc.vector.memset(cgT[0][:], 0.0)
            nc.vector.memset(x_nat[:, :, E:], 0.0)  # pad cols only: gather writes [:E]
            nc.vector.memset(gg_sb[:].bitcast(F32), 0.0)
            # constant rows BL..2BL-1 of the fi columns: scaled 1.0 so the
            # msel mask-penalty rows push masked-position fi logits to -30
            # (DMA: engines cannot address a base partition of 4)
            nc.sync.dma_start(gg_sb[BL : 2 * BL, 7 * H :], pen_d.ap())
            nc.vector.memset(xT8[:, EC:], 0.0)  # fp8 pad chunk (rows 384:512)

            # ---------------- prologue: embedding gather + transpose ----------------
            for e in range(BL):
                nc.gpsimd.indirect_dma_start(
                    out=x_nat[:, e, :E],
                    out_offset=None,
                    in_=embed_d.ap(),
                    in_offset=bass.IndirectOffsetOnAxis(ap=idx_sb[:, e : e + 1], axis=0),
                )
            for e in range(BL):
                for ec in range(EC):
                    pst = psumB.tile([128, 128], F32, name="pst", tag="pB")
                    nc.tensor.transpose(
                        pst[:], x_nat[:, e, ec * 128 : (ec + 1) * 128], ident_sb[:]
                    )
                    nc.scalar.copy(xT[:, ec, e * L : (e + 1) * L], pst[:])
                    nc.scalar.activation(
                        xT8[:, ec, e * L : (e + 1) * L], pst[:], AF.Identity, scale=SX
                    )

            # ---------------- prologue: h0 = tanh(x@W0+b0)*mask, g0 ----------------
            for hk in range(HC):
                w0p = p_wx.tile([128, EC, 128], F32R, name="w0p", tag="wx")
                nc.sync.dma_start(w0p[:], w0_d.ap()[hk])
                ps = psumA.tile([128, N], F32, name="ps_h0", tag="pA")
                for kc in range(EC):
                    nc.tensor.matmul(
                        ps[:], w0p[:, kc], xT[:, kc],
                        start=(kc == 0), stop=(kc == EC - 1),
                    )
                h0t = tmp2("h0t", F32)
                nc.scalar.activation(h0t[:], ps[:], AF.Tanh, bias=b0_sb[:, hk : hk + 1])
                nc.vector.scalar_tensor_tensor(
                    out=hD[0][:, hk], in0=h0t[:], scalar=SH,
                    in1=mask_sb[:], op0=AL.mult, op1=AL.mult,
                )
                emit_shift_dmas(hD[0], hL[0], hR[0], hk)
                hsum = sm("hsum")
                nc.vector.reduce_sum(hsum[:], v3a(hD[0][:, hk]), axis=AX.X)
                nc.vector.scalar_tensor_tensor(
                    out=gT[0][:, hk], in0=hsum[:], scalar=1.0 / SH,
                    in1=invlen_sb[:], op0=AL.mult, op1=AL.mult,
                )

            # ---------------- prologue: gate_x = (x @ Wg_x) * SH*SW ----------------
            # kept at psum scale in bf16; each step's chunk psum is seeded with
            # it by a single identity matmul, so no per-step x GEMM or evict add
            for grp in range(4):
                wxp = p_wx.tile([128, GC // 4, EC4, 128], FP8, name="wxp", tag="wx")
                nc.sync.dma_start(wxp[:], wg_x.ap()[grp])
                for mm in range(GC // 4):
                    m = grp * (GC // 4) + mm
                    ps = psumA.tile([128, N], F32, name="ps_gx", tag="pA")
                    for kp in range(EC4 // 2):
                        nc.tensor.matmul(
                            ps[:], wxp[:, mm, 2 * kp : 2 * kp + 2],
                            xT8[:, 2 * kp : 2 * kp + 2],
                            start=(kp == 0), stop=(kp == EC4 // 2 - 1),
                            perf_mode=DR,
                        )
                    nc.scalar.activation(gate_x[:, m], ps[:], AF.Identity)

            # ---------------- steps ----------------
            for s in range(NUM_STEPS):
                cur, nxt = s % 2, (s + 1) % 2
                h_c, h_n = hD[cur], hD[nxt]
                hl_c, hr_c = hL[cur], hR[cur]
                hl_n, hr_n = hL[nxt], hR[nxt]
                c_c, c_n = cD[cur], cD[nxt]
                cl_c, cr_c = cL[cur], cR[cur]
                cl_n, cr_n = cL[nxt], cR[nxt]
                g_c, g_n = gT[cur], gT[nxt]
                cg_c, cg_n = cgT[cur], cgT[nxt]

                def emit_gg(g_c=g_c):
                    # gg[0:BL] = g @ [Wg_g | Wfi_g]; g stationary, weights
                    # moving; one merged DMA per column group, produced in
                    # the order the big-GEMM selectors consume them
                    for nj in (6, 0, 1, 2, 3, 4, 5, 7):
                        wcp = p_wcat.tile([128, HC, 512], BF16, name="wcp", tag="wc")
                        nc.sync.dma_start(wcp[:], wg_gcat.ap()[nj])
                        psg = psumB.tile([BL, 512], F32, name="psg", tag="pB")
                        for kc in range(HC):
                            nc.tensor.matmul(
                                psg[:], g_c[:, kc], wcp[:, kc],
                                start=(kc == 0), stop=(kc == HC - 1),
                            )
                        nc.scalar.copy(gg_sb[0:BL, nj * 512 : (nj + 1) * 512], psg[:])

                def emit_hmm(m, shift_tiles=(hl_c, h_c, hr_c)):
                    wp = p_wg.tile([128, KHH, 128], FP8, name="wp", tag="wg")
                    nc.sync.dma_start(wp[:], wg_full.ap()[m])
                    ps = psumA.tile([128, N], F32, name="ps_g", tag="pA")
                    # seed the psum with the precomputed (already scaled) x part
                    nc.tensor.matmul(
                        ps[:], identb_sb[:], gate_x[:, m], start=True, stop=False
                    )
                    for kp in range(KHH // 2):
                        t = shift_tiles[kp // 2]  # 0: h_{i-1}, 1: h_i, 2: h_{i+1}
                        q = (kp % 2) * 2
                        nc.tensor.matmul(
                            ps[:], wp[:, 2 * kp : 2 * kp + 2], t[:, q : q + 2],
                            start=False, stop=False, perf_mode=DR,
                        )
                    return ps

                def emit_sel_evict(m, ps, j):
                    nc.tensor.matmul(
                        ps[:], gg_sb[:, m * 128 : (m + 1) * 128], sel_sb[:],
                        start=False, stop=True,
                    )
                    # eviction is a single Scalar activation: descale + bias
                    # + nonlinearity, PSUM -> bf16 SBUF
                    et = p_gate.tile([128, N], BF16, name=f"eg{j}", tag="gate")
                    fn = AF.Exp if j < 5 else (AF.Sigmoid if j == 5 else AF.Tanh)
                    nc.scalar.activation(
                        et[:], ps[:], fn, bias=bg_sb[:, m : m + 1], scale=DS
                    )
                    return et

                J_ORDER = (5, 0, 1, 2, 3, 4, 6)  # o first, exps, u last
                h_avg = []
                pend_shifts = []
                for hk in range(HC):
                    eg = {}
                    held = []
                    for idx, j in enumerate(J_ORDER):
                        m = j * HC + hk
                        ps = emit_hmm(m)
                        if hk == 0 and idx < 3:
                            # hold the first chunks' sel+evict until the gg
                            # GEMM is in the PE queue: PE then has ~3 chunks
                            # of h/x matmul runway across the step boundary
                            # while the slot-softmax tail produces g_n.
                            held.append((m, ps, j))
                            continue
                        if hk == 0 and idx == 3:
                            emit_gg()
                            for hm, hps, hj in held:
                                eg[hj] = emit_sel_evict(hm, hps, hj)
                            # replay the recurrence ops skipped while holding
                            s01 = tmp2("s01")
                            nc.gpsimd.tensor_add(s01[:], eg[0][:], eg[1][:])
                            t1 = tmp2("t1")
                            nc.vector.tensor_mul(t1[:], eg[1][:], cl_c[:, hk])
                        eg[j] = emit_sel_evict(m, ps, j)
                        # emit recurrence ops as soon as inputs exist; all bf16
                        # so DVE runs 2x-packed. None of these gate PSUM reuse.
                        if idx == 2:
                            s01 = tmp2("s01")
                            nc.gpsimd.tensor_add(s01[:], eg[0][:], eg[1][:])
                            t1 = tmp2("t1")
                            nc.vector.tensor_mul(t1[:], eg[1][:], cl_c[:, hk])
                        elif idx == 3:
                            t2 = tmp2("t2")
                            nc.gpsimd.tensor_mul(t2[:], eg[2][:], c_c[:, hk])
                        elif idx == 4:
                            s23 = tmp2("s23")
                            nc.gpsimd.tensor_add(s23[:], eg[2][:], eg[3][:])
                            t3 = tmp2("t3")
                            nc.vector.tensor_mul(t3[:], eg[3][:], cr_c[:, hk])
                            p12 = tmp2("p12")
                            nc.vector.tensor_add(p12[:], t1[:], t2[:])
                            s03 = tmp2("s03")
                            nc.gpsimd.tensor_add(s03[:], s01[:], s23[:])
                        elif idx == 5:
                            S5 = tmp2("S5", F32)
                            nc.vector.tensor_add(S5[:], s03[:], eg[4][:])
                            r5 = tmp2("r5", F32)
                            nc.vector.reciprocal_approx_fast(r5[:], S5[:])
                            rm = tmp2("rm")
                            nc.vector.tensor_mul(rm[:], r5[:], mask_sb[:])
                            # p34 = t3 + Es*cg, fused per example
                            p34 = tmp3("p34")
                            es3 = v3(eg[4])
                            t33 = v3(t3)
                            for e in range(BL):
                                nc.vector.scalar_tensor_tensor(
                                    out=p34[:, e], in0=es3[:, e],
                                    scalar=cg_c[:, hk, e : e + 1], in1=t33[:, e],
                                    op0=AL.mult, op1=AL.add,
                                )
                            macc = tmp2("macc")
                            nc.vector.tensor_add(
                                macc[:], p12[:],
                                p34[:].rearrange("p e l -> p (e l)"),
                            )
                        elif idx == 6:
                            m1 = tmp2("m1")
                            nc.vector.tensor_mul(m1[:], eg[0][:], eg[6][:])
                            pre = tmp2("pre")
                            nc.vector.tensor_add(pre[:], macc[:], m1[:])
                            nc.vector.tensor_mul(
                                out=c_n[:, hk], in0=pre[:], in1=rm[:]
                            )
                            tanh_c = tmp2("tanh_c")
                            nc.scalar.activation(tanh_c[:], c_n[:, hk], AF.Tanh)
                    # after the o gate: h_new (fp8, scaled by SH) and shadows
                    nc.vector.scalar_tensor_tensor(
                        out=h_n[:, hk], in0=eg[5][:], scalar=SH,
                        in1=tanh_c[:], op0=AL.mult, op1=AL.mult,
                    )
                    pend_shifts.append(hk)
                    hsum = sm("hsum2")
                    nc.vector.reduce_sum(hsum[:], v3a(h_n[:, hk]), axis=AX.X)
                    hav = p_small.tile([128, BL], BF16, name="hav", tag="sm")
                    nc.vector.scalar_tensor_tensor(
                        out=hav[:], in0=hsum[:], scalar=1.0 / SH,
                        in1=invlen_sb[:], op0=AL.mult, op1=AL.mult,
                    )
                    h_avg.append(hav)

                # -- fi GEMM: second half deferred so PE has ready work while
                # h_n[3] finishes
                efims = []
                psfs = []
                for hk in range(HC):
                    psf = psumA.tile([128, N], F32, name="psf", tag="pA")
                    nc.tensor.matmul(
                        psf[:], wfi_sb[:, 0:2, hk * 128 : (hk + 1) * 128],
                        h_n[:, 0:2],
                        start=True, stop=False, perf_mode=DR,
                    )
                    nc.tensor.matmul(
                        psf[:],
                        gg_sb[:, 7 * H + hk * 128 : 7 * H + (hk + 1) * 128],
                        msel_sb[:],
                        start=False, stop=False,
                    )
                    psfs.append(psf)
                # keep-warm punctuation: tiny throwaway matmuls that depend on
                # successive points of the hk=3 recurrence chain so the PE
                # activity monitor never sees a full idle window.
                for dep in (s03[:, :128], S5[:].bitcast(BF16)[:, :128], rm[:, :128],
                            macc[:, :128], tanh_c[:, :128]):
                    dmy = psumB.tile([64, 128], F32, name="dmy", tag="pB")
                    nc.tensor.matmul(
                        dmy[:, : dep.free_size()], mask_sb[:, :64], dep,
                        start=True, stop=True,
                    )
                ssum_t = p_small.tile([128, HC, BL], F32, name="ssum_t", tag="st")
                for hk in range(HC):
                    psf = psfs[hk]
                    nc.tensor.matmul(
                        psf[:], wfi_sb[:, 2:4, hk * 128 : (hk + 1) * 128],
                        h_n[:, 2:4],
                        start=False, stop=True, perf_mode=DR,
                    )
                    # masked positions carry -30 from the msel penalty rows, so
                    # exp() is ~0 there and the per-example accum_out IS ssum
                    efi = tmp2("efi")
                    ef3 = v3(efi)
                    pf3 = v3a(psf[:])
                    for e in range(BL):
                        nc.scalar.activation(
                            ef3[:, e], pf3[:, e], AF.Exp,
                            bias=bfi_sb[:, hk : hk + 1], scale=DS,
                            accum_out=ssum_t[:, hk, e : e + 1],
                        )
                    efims.append(efi)

                # -- fg / og GEMMs (transposed, N=4), in m-chunk pairs with the
                # h_avg[3] contribution deferred to keep PE fed
                fo_t = p_small.tile([128, 2 * HC, BL], F32, name="fo_t", tag="st")
                for pair in range(HC):
                    wfp = p_wgfgo.tile(
                        [128, 2 * HC, 256], BF16, name="wfp", tag="wf"
                    )
                    nc.sync.dma_start(wfp[:], wfo_d.ap()[pair])
                    mos = (2 * pair, 2 * pair + 1)
                    psts = []
                    for half, mo in enumerate(mos):
                        pst = psumT.tile([128, BL], F32, name="pst_f", tag="pT")
                        for kc in range(2 * HC - 1):
                            rhs = g_c[:, kc] if kc < HC else h_avg[kc - HC][:]
                            nc.tensor.matmul(
                                pst[:],
                                wfp[:, kc, half * 128 : (half + 1) * 128], rhs,
                                start=(kc == 0), stop=False,
                            )
                        psts.append(pst)
                    for half, (mo, pst) in enumerate(zip(mos, psts)):
                        mm = mo % HC
                        nc.tensor.matmul(
                            pst[:], wfp[:, 2 * HC - 1, half * 128 : (half + 1) * 128],
                            h_avg[HC - 1][:],
                            start=False, stop=True,
                        )
                        if mo < HC:
                            nc.scalar.activation(
                                fo_t[:, mo], pst[:], AF.Exp,
                                bias=bgf_sb[:, mm : mm + 1],
                            )
                        else:
                            nc.scalar.activation(
                                fo_t[:, mo], pst[:], AF.Sigmoid,
                                bias=bgo_sb[:, mm : mm + 1],
                            )

                # -- slot softmax + cg/g update: wide ops per hk, then the
                # tiny per-(hk,example) chain batched into single [128,16] ops
                s_c_t = p_small.tile([128, HC, BL], F32, name="s_c_t", tag="st")
                for hk in range(HC):
                    pw = tmp2("pw")
                    nc.vector.tensor_mul(pw[:], efims[hk][:], c_n[:, hk])
                    nc.vector.reduce_sum(s_c_t[:, hk], v3a(pw[:]), axis=AX.X)
                efg_t = fo_t[:, 0:HC]
                ogs_t = fo_t[:, HC:]
                den_t = p_small.tile([128, HC, BL], F32, name="den_t", tag="st")
                nc.vector.tensor_add(den_t[:], efg_t, ssum_t[:])
                rden_t = p_small.tile([128, HC, BL], F32, name="rden_t", tag="st")
                nc.vector.reciprocal(rden_t[:], den_t[:])
                num_t = p_small.tile([128, HC, BL], F32, name="num_t", tag="st")
                nc.vector.tensor_mul(num_t[:], efg_t, cg_c[:])
                nc.vector.tensor_add(num_t[:], num_t[:], s_c_t[:])
                nc.vector.tensor_mul(out=cg_n[:], in0=num_t[:], in1=rden_t[:])
                tcg_t = p_small.tile([128, HC, BL], F32, name="tcg_t", tag="st")
                nc.scalar.activation(tcg_t[:], cg_n[:], AF.Tanh)
                nc.vector.tensor_mul(out=g_n[:], in0=ogs_t, in1=tcg_t[:])
                # shift DMAs issued here (sync queue): all h_n/c_n chunks are
                # written by now, so no head-of-line wait blocks weight loads
                if s < NUM_STEPS - 1:
                    for hk in pend_shifts:
                        for td, tl, tr in ((h_n, hl_n, hr_n), (c_n, cl_n, cr_n)):
                            d3 = v3a(td[:, hk])
                            nc.gpsimd.dma_start(
                                out=v3a(tl[:, hk])[:, :, 1:L],
                                in_=d3[:, :, 0 : L - 1],
                            )
                            nc.gpsimd.dma_start(
                                out=v3a(tr[:, hk])[:, :, 0 : L - 1],
                                in_=d3[:, :, 1:L],
                            )

            # ---------------- epilogue ----------------
            g_fin = gT[NUM_STEPS % 2]
            for mo in range(2 * HC):
                pst = psumT.tile([128, BL], F32, name="pst_a1", tag="pT")
                for kc in range(HC):
                    nc.tensor.matmul(
                        pst[:], w1_sb[:, mo, kc], g_fin[:, kc],
                        start=(kc == 0), stop=(kc == HC - 1),
                    )
                nc.scalar.activation(
                    a1T[:, mo], pst[:], AF.Tanh, bias=b1_sb[:, mo : mo + 1]
                )

            pslg = psumB.tile([BL, DP], F32, name="pslg", tag="pB")
            for kc in range(2 * HC):
                nc.tensor.matmul(
                    pslg[:], a1T[:, kc], w2_sb[:, kc],
                    start=(kc == 0), stop=(kc == 2 * HC - 1),
                )
            lg = p_small.tile([BL, DP], F32, name="lg", tag="lg")
            nc.vector.tensor_add(lg[:], pslg[:], b2_sb[:])
            mx = p_small.tile([BL, 1], F32, name="mx", tag="lg")
            nc.vector.reduce_max(mx[:], lg[:, :DOUT], axis=AX.X)
            tsh = p_small.tile([BL, DOUT], F32, name="tsh", tag="lg")
            nc.vector.tensor_scalar(tsh[:], lg[:, :DOUT], mx[:], None, AL.subtract)
            ex = p_small.tile([BL, DOUT], F32, name="ex", tag="lg")
            ssum = p_small.tile([BL, 1], F32, name="ssum_l", tag="lg")
            nc.scalar.activation(ex[:], tsh[:], AF.Exp, accum_out=ssum[:])
            lse = p_small.tile([BL, 1], F32, name="lse", tag="lg")
            nc.scalar.activation(lse[:], ssum[:], AF.Ln)
            res = p_small.tile([BL, DOUT], F32, name="res", tag="lg")
            nc.vector.tensor_scalar(res[:], tsh[:], lse[:], None, AL.subtract)
            nc.sync.dma_start(out_d.ap(), res[:])

    nc.compile()
    return nc


def prep_in_maps(inputs):
    """Host-side prep: slice per core, pad/retile/quantize weights."""
    tokens = np.asarray(inputs["tokens"]).astype(np.int32)
    lengths = np.asarray(inputs["lengths"]).astype(np.int32)
    f = lambda k: np.ascontiguousarray(np.asarray(inputs[k], dtype=np.float32))
    embed = f("embed")
    W0, b0 = f("W0"), f("b0")
    Wg, bg = f("Wg"), f("bg")
    Wgf, bgf = f("Wgf"), f("bgf")
    Wfi, bfi = f("Wfi"), f("bfi")
    Wgo, bgo = f("Wgo"), f("bgo")
    W1, b1 = f("W1"), f("b1")
    W2, b2 = f("W2"), f("b2")

    def tile_km(w, kc, mc):
        # [kc*128, mc*128] -> [mc, 128, kc, 128]: piece[m][p,k,c] = w[k*128+p, m*128+c]
        return np.ascontiguousarray(
            w.reshape(kc, 128, mc, 128).transpose(2, 1, 0, 3)
        )

    f8 = ml_dtypes.float8_e4m3

    def q8(w, scale):
        return np.clip(w * scale, -240.0, 240.0).astype(f8)

    wg_full = q8(tile_km(Wg[: 3 * H], KHH, GC), SW)
    wg_x_pad = np.zeros((EP4, 7 * H), np.float32)
    wg_x_pad[:E] = Wg[3 * H : 3 * H + E]
    wg_x = q8(
        np.ascontiguousarray(
            tile_km(wg_x_pad, EC4, GC).reshape(4, GC // 4, 128, EC4, 128)
            .transpose(0, 2, 1, 3, 4)
        ),
        SWX,
    )

    # gg weights scaled so gg lands at the psum scale SH*SW after the fp8
    # GEMM against g8 (= g * SG): SG * SWCAT == SH * SW
    gcat = np.concatenate([Wg[3 * H + E :], Wfi[:H]], axis=1) * (SH * SW)
    wg_gcat = np.ascontiguousarray(
        gcat.reshape(HC, 128, GGC, 512).transpose(2, 1, 0, 3)
    )
    wfi_hp = q8(
        np.ascontiguousarray(Wfi[H:].reshape(HC, 128, H).transpose(1, 0, 2)), SW
    )
    # [Wgf | Wgo] with adjacent m-chunk pairs side by side: piece[g][p, k, c]
    wgf_t = Wgf.reshape(2 * HC, 128, HC, 128).transpose(2, 1, 0, 3)  # [mc,128,kc,128]
    wgo_t = Wgo.reshape(2 * HC, 128, HC, 128).transpose(2, 1, 0, 3)
    wfo = np.concatenate(
        [
            np.concatenate([wgf_t[0], wgf_t[1]], axis=-1)[None],
            np.concatenate([wgf_t[2], wgf_t[3]], axis=-1)[None],
            np.concatenate([wgo_t[0], wgo_t[1]], axis=-1)[None],
            np.concatenate([wgo_t[2], wgo_t[3]], axis=-1)[None],
        ],
        axis=0,
    )  # [4, 128, 2HC, 256]
    w0_pad = np.zeros((EP, H), np.float32)
    w0_pad[:E] = W0
    w0 = tile_km(w0_pad, EC, HC)
    w1 = np.ascontiguousarray(W1.reshape(HC, 128, 2 * HC, 128).transpose(1, 2, 0, 3))
    w2p = np.zeros((2 * H, DP), np.float32)
    w2p[:, :DOUT] = W2
    b2p = np.zeros((DP,), np.float32)
    b2p[:DOUT] = b2
    w2 = np.ascontiguousarray(w2p.reshape(2 * HC, 128, DP).transpose(1, 0, 2))

    def t_bias(b):
        return np.ascontiguousarray(b.reshape(-1, 128).T)

    sel = np.zeros((128, N), np.float32)
    for e in range(BL):
        sel[e, e * L : (e + 1) * L] = 1.0
    ident = np.eye(128, dtype=np.float32)

    bf = ml_dtypes.bfloat16
    shared = dict(
        embed=embed, wg_full=wg_full, wg_x=wg_x,
        wg_gcat=wg_gcat.astype(bf), wfi_h=wfi_hp,
        wfo=np.ascontiguousarray(wfo).astype(bf), w0=w0,
        w1=w1.astype(bf), w2=w2.astype(bf),
        bg_t=t_bias(bg), b0_t=t_bias(b0), bfi_t=t_bias(bfi), bgf_t=t_bias(bgf),
        bgo_t=t_bias(bgo), b1_t=t_bias(b1),
        b2_r=np.ascontiguousarray(np.tile(b2p[None, :], (BL, 1))),
        sel=sel, ident=ident, ident_bf=ident.astype(bf),
        pen_rows=np.full((BL, H), SH * SW, np.float32),
    )

    in_maps = []
    for c in range(NCORES):
        sl = slice(c * BL, (c + 1) * BL)
        tok = tokens[sl]                                   # [BL, L]
        lens = np.maximum(lengths[sl].astype(np.float32), 1.0)
        mask = (np.arange(L)[None, :] < lengths[sl][:, None]).astype(np.float32)
        mask_rep = np.ascontiguousarray(
            np.broadcast_to(mask.reshape(1, N), (128, N))
        ).astype(bf)
        invlen_rep = np.ascontiguousarray(
            np.broadcast_to((1.0 / lens).reshape(1, BL), (128, BL))
        )
        tok_idx = np.ascontiguousarray(tok.T.astype(np.int32))  # [L=128, BL]
        msel = sel.copy()
        for e in range(BL):
            msel[BL + e, e * L : (e + 1) * L] = -30.0 * (1.0 - mask[e])
        m = dict(shared)
        m.update(tok_idx=tok_idx, mask_rep=mask_rep, invlen_rep=invlen_rep,
                 msel=msel)
        in_maps.append(m)
    return in_maps


_NC_CACHE = {}


def kernel(**inputs) -> np.ndarray:
    in_maps = prep_in_maps(inputs)
    if "nc" not in _NC_CACHE:
        _NC_CACHE["nc"] = build_nc()
    nc = _NC_CACHE["nc"]
    res = run_bass_kernel_spmd(nc, in_maps, core_ids=list(range(NCORES)))
    return np.concatenate([r["out"] for r in res.results], axis=0)


if __name__ == "__main__":
    nc = build_nc()
    print("built ok")



# revision 20
# speedup vs baseline: 1.2037x; 1.2037x over previous
"""Trainium2 Bass kernel for the S-LSTM (sentence-state LSTM) classifier.

Data-parallel over batch: 8 cores x 4 examples. Everything on-chip runs in a
"transposed" layout: feature channels on SBUF partitions, (example, position)
flattened on the free dim (4*128 = 512 columns). The per-step gate GEMM runs
in fp8e4m3 DoubleRow mode (256 K-rows per 512-cycle matmul): the h state is
stored as fp8 scaled by SH, weights are pre-scaled by SW on the host, and the
descale folds into the eviction activation's scale parameter.

Position shifts (h_{i-1}, h_{i+1}, c shifts) are materialized into dense
shadow tiles (hL/hR, cL/cR) by SBUF->SBUF DMAs issued from the Scalar HWDGE
queue; per-example boundary zero columns are written once at init. This keeps
every DoubleRow moving operand a clean 3-D [128, 2, N] view and every
elementwise operand contiguous 16-bit (DVE 2x packing).

The x-part of the gates (x @ Wg_x, constant across steps) is folded into the
same psum accumulation as 2 extra DoubleRow matmuls per chunk (the x weight
chunks ride in the same per-chunk weight DMA), so gate eviction is a single
Scalar activation(Exp/Sigmoid/Tanh, bias=bg, scale=descale) reading PSUM --
no Vector op gates PSUM recycling.

The global-node ("g") part of ctx is rank-1 along positions: gg = g @ Wg_g is
computed once per step as a tiny M=4 fp8-DoubleRow GEMM, then folded into the
big GEMM as an extra K chunk against a constant 0/1 selector matrix.
"""

import ml_dtypes
import numpy as np

import concourse.bass as bass
import concourse.mybir as mybir
from concourse import bacc
import concourse.tile as tile
from concourse.bass_utils import run_bass_kernel_spmd

F32 = mybir.dt.float32
F32R = mybir.dt.float32r
F16 = mybir.dt.float16
BF16 = mybir.dt.bfloat16
FP8 = mybir.dt.float8e4
I32 = mybir.dt.int32
AL = mybir.AluOpType
AF = mybir.ActivationFunctionType
AX = mybir.AxisListType
DR = mybir.MatmulPerfMode.DoubleRow

B, L, V, E, H, DOUT = 32, 128, 30000, 300, 512, 5
NUM_STEPS = 5
NCORES = 8
BL = B // NCORES          # 4 examples per core
N = BL * L                # 512 free columns
EP = 384                  # E padded to 3*128 (f32r h0 GEMM)
HC = H // 128             # 4 H chunks
GC = 7 * H // 128         # 28 gate output chunks
KHH = 3 * HC              # 12 K chunks for hl/h/hr
EC = EP // 128            # 3 E chunks
EP4 = 512                 # E padded to 4*128 (fp8 DoubleRow pairs)
EC4 = EP4 // 128          # 4 E chunks for the fp8 x-part
KT = KHH + EC4            # 16 K chunks in the combined per-chunk weight tile
GG_W = 7 * H + H          # 4096: [Wg_g | Wfi_g] columns
GGC = GG_W // 512         # 8
DP = 8                    # DOUT padded to even size for fp32r matmul

# fp8 scales (powers of two; descales fold into activation scale params).
# SH*SW == SX*SWX == SG*SWCAT so every contribution to a gate psum carries the
# same scale and one descale applies.
SH = 16.0                 # h state scale (|h| <= 1 -> <= 16)
SW = 2048.0               # Wg/Wfi weight scale (absmax ~0.11 -> <= 222 < 240)
SX = 16.0                 # embedding scale (absmax ~0.52 -> <= 8.4)
SWX = 2048.0              # Wg_x weight scale
SG = 16.0                 # g state scale (|g| <= 1)
SWCAT = 2048.0            # Wg_gcat/Wfi_g weight scale
SWF8 = 2048.0             # Wgf/Wgo weight scale
DS = 1.0 / (SH * SW)      # uniform descale for the fp8 GEMM psums
DSF = 1.0 / (SG * SWF8)   # descale for the fo (fg/og) fp8 GEMM


def build_nc():
    nc = bacc.Bacc(trn_type="TRN2", target_bir_lowering=False)

    d = {}

    def din(name, shape, dt=F32):
        d[name] = nc.dram_tensor(name, list(shape), dt, kind="ExternalInput")
        return d[name]

    # weights are host-pre-tiled so every DMA reads contiguous HBM
    embed_d = din("embed", (V, E))
    wg_full = din("wg_full", (GC, 128, KHH, 128), FP8)  # h/hl/hr K chunks
    wg_x = din("wg_x", (4, 128, GC // 4, EC4, 128), FP8)
    wcat8_d = din("wcat8", (128, HC, GG_W), FP8)        # [Wg_g | Wfi_g], resident
    wfi_h = din("wfi_h", (128, HC, H), FP8)
    wfo8_d = din("wfo8", (128, HC, 2, 2 * HC, 128), FP8)  # [Wgf | Wgo], resident
    w0_d = din("w0", (HC, 128, EC, 128), F32R)
    w1_d = din("w1", (128, 2 * HC, HC, 128), BF16)
    w2_d = din("w2", (128, 2 * HC, DP), BF16)
    bg_t = din("bg_t", (128, GC))
    b0_t = din("b0_t", (128, HC))
    bfi_t = din("bfi_t", (128, HC))
    bgf_t = din("bgf_t", (128, HC))
    bgo_t = din("bgo_t", (128, HC))
    b1_t = din("b1_t", (128, 2 * HC))
    b2_r = din("b2_r", (BL, DP))
    sel_d = din("sel", (128, N), F32R)
    msel_d = din("msel", (128, N), F32R)
    pen_d = din("pen_rows", (BL, H), F32R)
    ident_d = din("ident", (128, 128))
    identb_d = din("ident_bf", (128, 128), BF16)
    tok_d = din("tok_idx", (128, BL), I32)       # column e = tokens of example e
    mask_d = din("mask_rep", (128, N), BF16)
    invlen_d = din("invlen_rep", (128, BL))

    out_d = nc.dram_tensor("out", [BL, DOUT], F32, kind="ExternalOutput")

    with tile.TileContext(nc) as tc:
        with (
            tc.tile_pool(name="psumA", bufs=4, space="PSUM") as psumA,
            tc.tile_pool(name="psumB", bufs=2, space="PSUM") as psumB,
            tc.tile_pool(name="psumT", bufs=2, space="PSUM") as psumT,
            tc.tile_pool(name="gates", bufs=10) as p_gate,
            tc.tile_pool(name="tmp", bufs=12) as p_tmp,
            tc.tile_pool(name="wg", bufs=12) as p_wg,
            tc.tile_pool(name="wx", bufs=3) as p_wx,
            tc.tile_pool(name="small", bufs=28) as p_small,
            tc.tile_pool(name="state", bufs=1) as p_state,
        ):
            # ---------------- persistent state ----------------
            def T(shape, name, dt=F32):
                return p_state.tile(shape, dt, name=name, tag=name)

            # h/c stored densely; +-1 position shifts live in shadow tiles
            # kept in sync by SBUF->SBUF DMAs (boundary zeros written once).
            hD = [T([128, HC, N], f"hD{i}", FP8) for i in range(2)]
            hL = [T([128, HC, N], f"hL{i}", FP8) for i in range(2)]
            hR = [T([128, HC, N], f"hR{i}", FP8) for i in range(2)]
            cD = [T([128, HC, N], f"cD{i}", BF16) for i in range(2)]
            cL = [T([128, HC, N], f"cL{i}", BF16) for i in range(2)]
            cR = [T([128, HC, N], f"cR{i}", BF16) for i in range(2)]
            gT = [T([128, HC, BL], f"gT{i}", BF16) for i in range(2)]
            cgT = [T([128, HC, BL], f"cgT{i}") for i in range(2)]
            xT = T([128, EC, N], "xT", F32R)
            xT8 = T([128, EC4, N], "xT8", FP8)
            gate_x = T([128, GC, N], "gate_x", BF16)
            # gg rows 0:BL hold g @ [Wg_g | Wfi_g]; rows BL:128 stay zero so the
            # selector matmul can contract over a full K=128.
            gg_sb = T([128, GG_W], "gg_sb", F32R)
            # gcat8: fp8 [g*SG | h_avg*SG] K rows for the DR gg / fo GEMMs;
            # inner dim padded to 16 (dual-fp8 LDWEIGHTS/matmul need the
            # K-pair dim step to be a multiple of 16 elements)
            gcat8 = T([128, 2 * HC, 16], "gcat8", FP8)
            wcat8_sb = T([128, HC, GG_W], "wcat8_sb", FP8)
            wfo8_sb = T([128, HC, 2, 2 * HC, 128], "wfo8_sb", FP8)
            x_nat = [T([128, EP], f"x_nat{e}") for e in range(BL)]
            idx_sb = T([128, BL], "idx_sb", I32)
            mask_sb = T([128, N], "mask_sb", BF16)
            invlen_sb = T([128, BL], "invlen_sb")
            sel_sb = T([128, N], "sel_sb", F32R)
            msel_sb = T([128, N], "msel_sb", F32R)
            ident_sb = T([128, 128], "ident_sb")
            identb_sb = T([128, 128], "identb_sb", BF16)
            wfi_sb = T([128, HC, H], "wfi_sb", FP8)
            w1_sb = T([128, 2 * HC, HC, 128], "w1_sb", BF16)
            w2_sb = T([128, 2 * HC, DP], "w2_sb", BF16)
            a1T = T([128, 2 * HC, BL], "a1T", BF16)
            bg_sb = T([128, GC], "bg_sb")
            b0_sb = T([128, HC], "b0_sb")
            bfi_sb = T([128, HC], "bfi_sb")
            bgf_sb = T([128, HC], "bgf_sb")
            bgo_sb = T([128, HC], "bgo_sb")
            b1_sb = T([128, 2 * HC], "b1_sb")
            b2_sb = T([BL, DP], "b2_sb")

            def mask3():
                return mask_sb[:].rearrange("p (e l) -> p e l", l=L)

            def v3(t):
                return t[:].rearrange("p (e l) -> p e l", l=L)

            def v3a(ap):
                return ap.rearrange("p (e l) -> p e l", l=L)

            def emit_shift_dmas(td, tl, tr, hk):
                # tl[i] = t[i-1], tr[i] = t[i+1]; per-example boundary cols
                # stay zero from the prologue memset. Issued from the Scalar
                # HWDGE queue so waits never block the Sync weight-load queue.
                d3 = v3a(td[:, hk])
                nc.scalar.dma_start(v3a(tl[:, hk])[:, :, 1:L], d3[:, :, 0 : L - 1])
                nc.scalar.dma_start(v3a(tr[:, hk])[:, :, 0 : L - 1], d3[:, :, 1:L])

            def tmp2(name, dt=BF16):
                return p_tmp.tile([128, N], dt, name=name, tag="tmp")

            def tmp3(name, dt=BF16):
                return p_tmp.tile([128, BL, L], dt, name=name, tag="tmp")

            def sm(name):
                return p_small.tile([128, BL], F32, name=name, tag="sm")

            # ---------------- prologue: loads ----------------
            # sync queue: step-0-critical smalls first (gathers need idx,
            # transposes need ident); bulky resident loads ride the scalar
            # HWDGE queue so they never delay the w0/wg_x/wg_full stream.
            nc.sync.dma_start(idx_sb[:], tok_d.ap())
            nc.sync.dma_start(ident_sb[:], ident_d.ap())
            nc.sync.dma_start(identb_sb[:], identb_d.ap())
            nc.sync.dma_start(mask_sb[:], mask_d.ap())
            nc.sync.dma_start(invlen_sb[:], invlen_d.ap())
            nc.sync.dma_start(sel_sb[:], sel_d.ap())
            nc.sync.dma_start(msel_sb[:], msel_d.ap())
            for t_sb, t_d in (
                (b0_sb, b0_t), (bg_sb, bg_t), (bfi_sb, bfi_t),
                (bgf_sb, bgf_t), (bgo_sb, bgo_t), (b1_sb, b1_t), (b2_sb, b2_r),
            ):
                nc.sync.dma_start(t_sb[:], t_d.ap())
            nc.scalar.dma_start(wcat8_sb[:], wcat8_d.ap())
            nc.scalar.dma_start(wfo8_sb[:], wfo8_d.ap())
            nc.scalar.dma_start(wfi_sb[:], wfi_h.ap())
            nc.scalar.dma_start(w1_sb[:], w1_d.ap())
            nc.scalar.dma_start(w2_sb[:], w2_d.ap())

            # ---------------- prologue: embedding gather + transpose ----------------
            for e in range(BL):
                nc.gpsimd.indirect_dma_start(
                    out=x_nat[e][:, :E],
                    out_offset=None,
                    in_=embed_d.ap(),
                    in_offset=bass.IndirectOffsetOnAxis(ap=idx_sb[:, e : e + 1], axis=0),
                )

            # zero only what a step actually reads before writing: buffer-0
            # c tiles (c0 = 0), shadow-tile boundary columns (written once,
            # never touched again), cg0, pads. Split across engines.
            nc.gpsimd.memset(cD[0][:], 0.0)
            nc.vector.memset(cL[0][:], 0.0)
            nc.gpsimd.memset(cR[0][:], 0.0)
            for t in (hL[1], hR[1], cL[1], cR[1]):
                bv = t[:].rearrange("p q (e l) -> p (q e) l", l=L)
                nc.vector.memset(bv[:, :, 0:1], 0.0)
                nc.vector.memset(bv[:, :, L - 1 : L], 0.0)
            for t in (hL[0], hR[0]):
                bv = t[:].rearrange("p q (e l) -> p (q e) l", l=L)
                nc.vector.memset(bv[:, :, 0:1], 0.0)
                nc.vector.memset(bv[:, :, L - 1 : L], 0.0)
            nc.vector.memset(cgT[0][:], 0.0)
            for e in range(BL):
                nc.vector.memset(x_nat[e][:, E:], 0.0)  # pad cols: gather writes [:E]
            nc.vector.memset(gg_sb[:].bitcast(F32), 0.0)
            # constant rows BL..2BL-1 of the fi columns: scaled 1.0 so the
            # msel mask-penalty rows push masked-position fi logits to -30
            # (DMA: engines cannot address a base partition of 4)
            nc.sync.dma_start(gg_sb[BL : 2 * BL, 7 * H :], pen_d.ap())
            nc.vector.memset(xT8[:, EC:], 0.0)  # fp8 pad chunk (rows 384:512)

            for e in range(BL):
                for ec in range(EC):
                    pst = psumB.tile([128, 128], F32, name="pst", tag="pB")
                    nc.tensor.transpose(
                        pst[:], x_nat[e][:, ec * 128 : (ec + 1) * 128], ident_sb[:]
                    )
                    nc.scalar.copy(xT[:, ec, e * L : (e + 1) * L], pst[:])
                    nc.scalar.activation(
                        xT8[:, ec, e * L : (e + 1) * L], pst[:], AF.Identity, scale=SX
                    )

            # ---------------- prologue: h0 = tanh(x@W0+b0)*mask, g0 ----------------
            for hk in range(HC):
                w0p = p_wx.tile([128, EC, 128], F32R, name="w0p", tag="wx")
                nc.sync.dma_start(w0p[:], w0_d.ap()[hk])
                ps = psumA.tile([128, N], F32, name="ps_h0", tag="pA")
                for kc in range(EC):
                    nc.tensor.matmul(
                        ps[:], w0p[:, kc], xT[:, kc],
                        start=(kc == 0), stop=(kc == EC - 1),
                    )
                h0t = tmp2("h0t", F32)
                nc.scalar.activation(h0t[:], ps[:], AF.Tanh, bias=b0_sb[:, hk : hk + 1])
                nc.vector.scalar_tensor_tensor(
                    out=hD[0][:, hk], in0=h0t[:], scalar=SH,
                    in1=mask_sb[:], op0=AL.mult, op1=AL.mult,
                )
                emit_shift_dmas(hD[0], hL[0], hR[0], hk)
                hsum = sm("hsum")
                nc.vector.reduce_sum(hsum[:], v3a(hD[0][:, hk]), axis=AX.X)
                nc.vector.scalar_tensor_tensor(
                    out=gT[0][:, hk], in0=hsum[:], scalar=1.0 / SH,
                    in1=invlen_sb[:], op0=AL.mult, op1=AL.mult,
                )
                nc.vector.scalar_tensor_tensor(
                    out=gcat8[:, hk, 0:BL], in0=hsum[:], scalar=SG / SH,
                    in1=invlen_sb[:], op0=AL.mult, op1=AL.mult,
                )

            # ---------------- prologue: gate_x = (x @ Wg_x) * SH*SW ----------------
            # kept at psum scale in bf16; each step's chunk psum is seeded with
            # it by a single identity matmul, so no per-step x GEMM or evict add
            for grp in range(4):
                wxp = p_wx.tile([128, GC // 4, EC4, 128], FP8, name="wxp", tag="wx")
                nc.sync.dma_start(wxp[:], wg_x.ap()[grp])
                for mm in range(GC // 4):
                    m = grp * (GC // 4) + mm
                    ps = psumA.tile([128, N], F32, name="ps_gx", tag="pA")
                    for kp in range(EC4 // 2):
                        nc.tensor.matmul(
                            ps[:], wxp[:, mm, 2 * kp : 2 * kp + 2],
                            xT8[:, 2 * kp : 2 * kp + 2],
                            start=(kp == 0), stop=(kp == EC4 // 2 - 1),
                            perf_mode=DR,
                        )
                    nc.scalar.activation(gate_x[:, m], ps[:], AF.Identity)

            # ---------------- steps ----------------
            for s in range(NUM_STEPS):
                cur, nxt = s % 2, (s + 1) % 2
                h_c, h_n = hD[cur], hD[nxt]
                hl_c, hr_c = hL[cur], hR[cur]
                hl_n, hr_n = hL[nxt], hR[nxt]
                c_c, c_n = cD[cur], cD[nxt]
                cl_c, cr_c = cL[cur], cR[cur]
                cl_n, cr_n = cL[nxt], cR[nxt]
                g_c, g_n = gT[cur], gT[nxt]
                cg_c, cg_n = cgT[cur], cgT[nxt]

                def emit_gg():
                    # gg[0:BL] = g @ [Wg_g | Wfi_g] as fp8 DoubleRow against
                    # the resident wcat8; produced in consumption order.
                    # Scales: (g*SG)(W*SWCAT) = raw*SH*SW, so a plain copy
                    # lands gg at the big-GEMM psum scale.
                    for nj in (5, 0, 1, 2, 3, 4, 6, 7):
                        psg = psumB.tile([BL, 512], F32, name="psg", tag="pB")
                        for i in range(HC // 2):
                            nc.tensor.matmul(
                                psg[:], gcat8[:, 2 * i : 2 * i + 2, 0:BL],
                                wcat8_sb[:, 2 * i : 2 * i + 2,
                                         nj * 512 : (nj + 1) * 512],
                                start=(i == 0), stop=(i == HC // 2 - 1),
                                perf_mode=DR,
                            )
                        nc.scalar.copy(gg_sb[0:BL, nj * 512 : (nj + 1) * 512], psg[:])

                def emit_hmm(m, shift_tiles=(h_c, hl_c, hr_c)):
                    wp = p_wg.tile([128, KHH, 128], FP8, name="wp", tag="wg")
                    nc.sync.dma_start(wp[:], wg_full.ap()[m])
                    ps = psumA.tile([128, N], F32, name="ps_g", tag="pA")
                    # seed the psum with the precomputed (already scaled) x part
                    nc.tensor.matmul(
                        ps[:], identb_sb[:], gate_x[:, m], start=True, stop=False
                    )
                    for kp in range(KHH // 2):
                        t = shift_tiles[kp // 2]  # 0: h_{i-1}, 1: h_i, 2: h_{i+1}
                        q = (kp % 2) * 2
                        nc.tensor.matmul(
                            ps[:], wp[:, 2 * kp : 2 * kp + 2], t[:, q : q + 2],
                            start=False, stop=False, perf_mode=DR,
                        )
                    return ps

                def emit_sel_evict(m, ps, j):
                    nc.tensor.matmul(
                        ps[:], gg_sb[:, m * 128 : (m + 1) * 128], sel_sb[:],
                        start=False, stop=True,
                    )
                    # eviction is a single Scalar activation: descale + bias
                    # + nonlinearity, PSUM -> bf16 SBUF
                    et = p_gate.tile([128, N], BF16, name=f"eg{j}", tag="gate")
                    fn = AF.Exp if j < 5 else (AF.Sigmoid if j == 5 else AF.Tanh)
                    nc.scalar.activation(
                        et[:], ps[:], fn, bias=bg_sb[:, m : m + 1], scale=DS
                    )
                    return et

                J_ORDER = (5, 0, 1, 2, 3, 4, 6)  # o first, exps, u last
                for hk in range(HC):
                    eg = {}
                    held = []
                    for idx, j in enumerate(J_ORDER):
                        m = j * HC + hk
                        ps = emit_hmm(m)
                        if hk == 0 and idx < 3:
                            # hold the first chunks' sel+evict until the gg
                            # GEMM is in the PE queue: PE then has ~3 chunks
                            # of h/x matmul runway across the step boundary
                            # while the slot-softmax tail produces g_n.
                            held.append((m, ps, j))
                            continue
                        if hk == 0 and idx == 3:
                            emit_gg()
                            for hm, hps, hj in held:
                                eg[hj] = emit_sel_evict(hm, hps, hj)
                            # replay the recurrence ops skipped while holding
                            s01 = tmp2("s01")
                            nc.gpsimd.tensor_add(s01[:], eg[0][:], eg[1][:])
                            t1 = tmp2("t1")
                            nc.vector.tensor_mul(t1[:], eg[1][:], cl_c[:, hk])
                        eg[j] = emit_sel_evict(m, ps, j)
                        # emit recurrence ops as soon as inputs exist; all bf16
                        # so DVE runs 2x-packed. None of these gate PSUM reuse.
                        if idx == 2:
                            s01 = tmp2("s01")
                            nc.gpsimd.tensor_add(s01[:], eg[0][:], eg[1][:])
                            t1 = tmp2("t1")
                            nc.vector.tensor_mul(t1[:], eg[1][:], cl_c[:, hk])
                        elif idx == 3:
                            t2 = tmp2("t2")
                            nc.gpsimd.tensor_mul(t2[:], eg[2][:], c_c[:, hk])
                        elif idx == 4:
                            s23 = tmp2("s23")
                            nc.gpsimd.tensor_add(s23[:], eg[2][:], eg[3][:])
                            t3 = tmp2("t3")
                            nc.vector.tensor_mul(t3[:], eg[3][:], cr_c[:, hk])
                            p12 = tmp2("p12")
                            nc.vector.tensor_add(p12[:], t1[:], t2[:])
                            s03 = tmp2("s03")
                            nc.gpsimd.tensor_add(s03[:], s01[:], s23[:])
                        elif idx == 5:
                            S5 = tmp2("S5", F32)
                            nc.vector.tensor_add(S5[:], s03[:], eg[4][:])
                            r5 = tmp2("r5", F32)
                            nc.vector.reciprocal_approx_fast(r5[:], S5[:])
                            rm = tmp2("rm")
                            nc.vector.tensor_mul(rm[:], r5[:], mask_sb[:])
                            # p34 = t3 + Es*cg, fused per example
                            p34 = tmp3("p34")
                            es3 = v3(eg[4])
                            t33 = v3(t3)
                            for e in range(BL):
                                nc.vector.scalar_tensor_tensor(
                                    out=p34[:, e], in0=es3[:, e],
                                    scalar=cg_c[:, hk, e : e + 1], in1=t33[:, e],
                                    op0=AL.mult, op1=AL.add,
                                )
                            macc = tmp2("macc")
                            nc.vector.tensor_add(
                                macc[:], p12[:],
                                p34[:].rearrange("p e l -> p (e l)"),
                            )
                        elif idx == 6:
                            m1 = tmp2("m1")
                            nc.vector.tensor_mul(m1[:], eg[0][:], eg[6][:])
                            pre = tmp2("pre")
                            nc.vector.tensor_add(pre[:], macc[:], m1[:])
                            nc.vector.tensor_mul(
                                out=c_n[:, hk], in0=pre[:], in1=rm[:]
                            )
                            tanh_c = tmp2("tanh_c")
                            nc.scalar.activation(tanh_c[:], c_n[:, hk], AF.Tanh)
                    # after the o gate: h_new (fp8, scaled by SH) and shadows
                    nc.vector.scalar_tensor_tensor(
                        out=h_n[:, hk], in0=eg[5][:], scalar=SH,
                        in1=tanh_c[:], op0=AL.mult, op1=AL.mult,
                    )
                    # shift DMAs issued per-hk (gpsimd queue) so next step's
                    # hl/hr shadows are ready with slack; K order in emit_hmm
                    # consumes the centre h first for extra headroom.
                    if s < NUM_STEPS - 1:
                        d3 = v3a(h_n[:, hk])
                        nc.gpsimd.dma_start(
                            out=v3a(hl_n[:, hk])[:, :, 1:L], in_=d3[:, :, 0 : L - 1]
                        )
                        nc.gpsimd.dma_start(
                            out=v3a(hr_n[:, hk])[:, :, 0 : L - 1], in_=d3[:, :, 1:L]
                        )
                        d3c = v3a(c_n[:, hk])
                        nc.gpsimd.dma_start(
                            out=v3a(cl_n[:, hk])[:, :, 1:L], in_=d3c[:, :, 0 : L - 1]
                        )
                        nc.gpsimd.dma_start(
                            out=v3a(cr_n[:, hk])[:, :, 0 : L - 1], in_=d3c[:, :, 1:L]
                        )
                    hsum = sm("hsum2")
                    nc.vector.reduce_sum(hsum[:], v3a(h_n[:, hk]), axis=AX.X)
                    # h_avg lands directly in the fp8 gcat8 K rows (scaled SG)
                    nc.vector.scalar_tensor_tensor(
                        out=gcat8[:, HC + hk, 0:BL], in0=hsum[:], scalar=SG / SH,
                        in1=invlen_sb[:], op0=AL.mult, op1=AL.mult,
                    )

                # -- fi GEMM: second half deferred so PE has ready work while
                # h_n[3] finishes
                efims = []
                psfs = []
                for hk in range(HC):
                    psf = psumA.tile([128, N], F32, name="psf", tag="pA")
                    nc.tensor.matmul(
                        psf[:], wfi_sb[:, 0:2, hk * 128 : (hk + 1) * 128],
                        h_n[:, 0:2],
                        start=True, stop=False, perf_mode=DR,
                    )
                    nc.tensor.matmul(
                        psf[:],
                        gg_sb[:, 7 * H + hk * 128 : 7 * H + (hk + 1) * 128],
                        msel_sb[:],
                        start=False, stop=False,
                    )
                    psfs.append(psf)
                # keep-warm punctuation: tiny throwaway matmuls that depend on
                # successive points of the hk=3 recurrence chain so the PE
                # activity monitor never sees a full idle window.
                for dep in (s03[:, :128], S5[:].bitcast(BF16)[:, :128], rm[:, :128],
                            macc[:, :128], tanh_c[:, :128]):
                    dmy = psumB.tile([64, 128], F32, name="dmy", tag="pB")
                    nc.tensor.matmul(
                        dmy[:, : dep.free_size()], mask_sb[:, :64], dep,
                        start=True, stop=True,
                    )
                ssum_t = p_small.tile([128, HC, BL], F32, name="ssum_t", tag="st")
                for hk in range(HC):
                    psf = psfs[hk]
                    nc.tensor.matmul(
                        psf[:], wfi_sb[:, 2:4, hk * 128 : (hk + 1) * 128],
                        h_n[:, 2:4],
                        start=False, stop=True, perf_mode=DR,
                    )
                    # masked positions carry -30 from the msel penalty rows, so
                    # exp() is ~0 there and the per-example accum_out IS ssum
                    efi = tmp2("efi")
                    ef3 = v3(efi)
                    pf3 = v3a(psf[:])
                    for e in range(BL):
                        nc.scalar.activation(
                            ef3[:, e], pf3[:, e], AF.Exp,
                            bias=bfi_sb[:, hk : hk + 1], scale=DS,
                            accum_out=ssum_t[:, hk, e : e + 1],
                        )
                    efims.append(efi)

                # -- fg / og GEMMs: fp8 DR against resident wfo8, K rows from
                # gcat8 = [g8 | hav8]; fg (Exp) chunks grouped before og
                # (Sigmoid) to spare activation-table reloads
                fo_t = p_small.tile([128, 2 * HC, BL], F32, name="fo_t", tag="st")
                for mo in range(2 * HC):
                    pst = psumT.tile([128, BL], F32, name="pst_f", tag="pT")
                    for kp in range(HC):
                        nc.tensor.matmul(
                            pst[:], wfo8_sb[:, kp, :, mo],
                            gcat8[:, 2 * kp : 2 * kp + 2, 0:BL],
                            start=(kp == 0), stop=(kp == HC - 1),
                            perf_mode=DR,
                        )
                    mm = mo % HC
                    if mo < HC:
                        nc.scalar.activation(
                            fo_t[:, mo], pst[:], AF.Exp,
                            bias=bgf_sb[:, mm : mm + 1], scale=DSF,
                        )
                    else:
                        nc.scalar.activation(
                            fo_t[:, mo], pst[:], AF.Sigmoid,
                            bias=bgo_sb[:, mm : mm + 1], scale=DSF,
                        )

                # -- slot softmax + cg/g update: wide ops per hk, then the
                # tiny per-(hk,example) chain batched into single [128,16] ops
                s_c_t = p_small.tile([128, HC, BL], F32, name="s_c_t", tag="st")
                for hk in range(HC):
                    pw = tmp2("pw")
                    nc.vector.tensor_mul(pw[:], efims[hk][:], c_n[:, hk])
                    nc.vector.reduce_sum(s_c_t[:, hk], v3a(pw[:]), axis=AX.X)
                efg_t = fo_t[:, 0:HC]
                ogs_t = fo_t[:, HC:]
                den_t = p_small.tile([128, HC, BL], F32, name="den_t", tag="st")
                nc.vector.tensor_add(den_t[:], efg_t, ssum_t[:])
                rden_t = p_small.tile([128, HC, BL], F32, name="rden_t", tag="st")
                nc.vector.reciprocal(rden_t[:], den_t[:])
                num_t = p_small.tile([128, HC, BL], F32, name="num_t", tag="st")
                nc.vector.tensor_mul(num_t[:], efg_t, cg_c[:])
                nc.vector.tensor_add(num_t[:], num_t[:], s_c_t[:])
                nc.vector.tensor_mul(out=cg_n[:], in0=num_t[:], in1=rden_t[:])
                tcg_t = p_small.tile([128, HC, BL], F32, name="tcg_t", tag="st")
                nc.scalar.activation(tcg_t[:], cg_n[:], AF.Tanh)
                nc.vector.tensor_mul(out=g_n[:], in0=ogs_t, in1=tcg_t[:])
                # fp8 copy of g for next step's DR gg/fo GEMMs
                nc.vector.tensor_scalar(
                    gcat8[:, 0:HC, 0:BL], g_n[:], SG, None, AL.mult
                )

            # ---------------- epilogue ----------------
            g_fin = gT[NUM_STEPS % 2]
            for mo in range(2 * HC):
                pst = psumT.tile([128, BL], F32, name="pst_a1", tag="pT")
                for kc in range(HC):
                    nc.tensor.matmul(
                        pst[:], w1_sb[:, mo, kc], g_fin[:, kc],
                        start=(kc == 0), stop=(kc == HC - 1),
                    )
                nc.scalar.activation(
                    a1T[:, mo], pst[:], AF.Tanh, bias=b1_sb[:, mo : mo + 1]
                )

            pslg = psumB.tile([BL, DP], F32, name="pslg", tag="pB")
            for kc in range(2 * HC):
                nc.tensor.matmul(
                    pslg[:], a1T[:, kc], w2_sb[:, kc],
                    start=(kc == 0), stop=(kc == 2 * HC - 1),
                )
            lg = p_small.tile([BL, DP], F32, name="lg", tag="lg")
            nc.vector.tensor_add(lg[:], pslg[:], b2_sb[:])
            mx = p_small.tile([BL, 1], F32, name="mx", tag="lg")
            nc.vector.reduce_max(mx[:], lg[:, :DOUT], axis=AX.X)
            tsh = p_small.tile([BL, DOUT], F32, name="tsh", tag="lg")
            nc.vector.tensor_scalar(tsh[:], lg[:, :DOUT], mx[:], None, AL.subtract)
            ex = p_small.tile([BL, DOUT], F32, name="ex", tag="lg")
            ssum = p_small.tile([BL, 1], F32, name="ssum_l", tag="lg")
            nc.scalar.activation(ex[:], tsh[:], AF.Exp, accum_out=ssum[:])
            lse = p_small.tile([BL, 1], F32, name="lse", tag="lg")
            nc.scalar.activation(lse[:], ssum[:], AF.Ln)
            res = p_small.tile([BL, DOUT], F32, name="res", tag="lg")
            nc.vector.tensor_scalar(res[:], tsh[:], lse[:], None, AL.subtract)
            nc.sync.dma_start(out_d.ap(), res[:])

    nc.compile()
    return nc


def prep_in_maps(inputs):
    """Host-side prep: slice per core, pad/retile/quantize weights."""
    tokens = np.asarray(inputs["tokens"]).astype(np.int32)
    lengths = np.asarray(inputs["lengths"]).astype(np.int32)
    f = lambda k: np.ascontiguousarray(np.asarray(inputs[k], dtype=np.float32))
    embed = f("embed")
    W0, b0 = f("W0"), f("b0")
    Wg, bg = f("Wg"), f("bg")
    Wgf, bgf = f("Wgf"), f("bgf")
    Wfi, bfi = f("Wfi"), f("bfi")
    Wgo, bgo = f("Wgo"), f("bgo")
    W1, b1 = f("W1"), f("b1")
    W2, b2 = f("W2"), f("b2")

    def tile_km(w, kc, mc):
        # [kc*128, mc*128] -> [mc, 128, kc, 128]: piece[m][p,k,c] = w[k*128+p, m*128+c]
        return np.ascontiguousarray(
            w.reshape(kc, 128, mc, 128).transpose(2, 1, 0, 3)
        )

    f8 = ml_dtypes.float8_e4m3

    def q8(w, scale):
        return np.clip(w * scale, -240.0, 240.0).astype(f8)

    # K-chunk order (h, hl, hr): emit_hmm consumes the centre h first so the
    # hl/hr shadow-shift DMAs get extra slack at step boundaries
    wg_full = q8(
        tile_km(Wg[: 3 * H], KHH, GC)[:, :, [4, 5, 6, 7, 0, 1, 2, 3, 8, 9, 10, 11]],
        SW,
    )
    wg_x_pad = np.zeros((EP4, 7 * H), np.float32)
    wg_x_pad[:E] = Wg[3 * H : 3 * H + E]
    wg_x = q8(
        np.ascontiguousarray(
            tile_km(wg_x_pad, EC4, GC).reshape(4, GC // 4, 128, EC4, 128)
            .transpose(0, 2, 1, 3, 4)
        ),
        SWX,
    )

    # gg weights fp8: gg = (g*SG) @ (W*SWCAT) lands at psum scale SH*SW
    # (SG*SWCAT == SH*SW), so the psg eviction is a plain copy
    gcat = np.concatenate([Wg[3 * H + E :], Wfi[:H]], axis=1)
    wcat8 = q8(
        np.ascontiguousarray(gcat.reshape(HC, 128, GG_W).transpose(1, 0, 2)), SWCAT
    )
    wfi_hp = q8(
        np.ascontiguousarray(Wfi[H:].reshape(HC, 128, H).transpose(1, 0, 2)), SW
    )
    # [Wgf | Wgo] fp8 for the DR fo GEMM: [128, kp, kw, mo, 128]
    wcat2 = np.concatenate([Wgf, Wgo], axis=1)  # [2H, 2H]
    wfo8 = q8(
        np.ascontiguousarray(
            wcat2.reshape(HC, 2, 128, 2 * HC, 128).transpose(2, 0, 1, 3, 4)
        ),
        SWF8,
    )
    w0_pad = np.zeros((EP, H), np.float32)
    w0_pad[:E] = W0
    w0 = tile_km(w0_pad, EC, HC)
    w1 = np.ascontiguousarray(W1.reshape(HC, 128, 2 * HC, 128).transpose(1, 2, 0, 3))
    w2p = np.zeros((2 * H, DP), np.float32)
    w2p[:, :DOUT] = W2
    b2p = np.zeros((DP,), np.float32)
    b2p[:DOUT] = b2
    w2 = np.ascontiguousarray(w2p.reshape(2 * HC, 128, DP).transpose(1, 0, 2))

    def t_bias(b):
        return np.ascontiguousarray(b.reshape(-1, 128).T)

    sel = np.zeros((128, N), np.float32)
    for e in range(BL):
        sel[e, e * L : (e + 1) * L] = 1.0
    ident = np.eye(128, dtype=np.float32)

    bf = ml_dtypes.bfloat16
    shared = dict(
        embed=embed, wg_full=wg_full, wg_x=wg_x,
        wcat8=wcat8, wfi_h=wfi_hp,
        wfo8=wfo8, w0=w0,
        w1=w1.astype(bf), w2=w2.astype(bf),
        bg_t=t_bias(bg), b0_t=t_bias(b0), bfi_t=t_bias(bfi), bgf_t=t_bias(bgf),
        bgo_t=t_bias(bgo), b1_t=t_bias(b1),
        b2_r=np.ascontiguousarray(np.tile(b2p[None, :], (BL, 1))),
        sel=sel, ident=ident, ident_bf=ident.astype(bf),
        pen_rows=np.full((BL, H), SH * SW, np.float32),
    )

    in_maps = []
    for c in range(NCORES):
        sl = slice(c * BL, (c + 1) * BL)
        tok = tokens[sl]                                   # [BL, L]
        lens = np.maximum(lengths[sl].astype(np.float32), 1.0)
        mask = (np.arange(L)[None, :] < lengths[sl][:, None]).astype(np.float32)
        mask_rep = np.ascontiguousarray(
            np.broadcast_to(mask.reshape(1, N), (128, N))
        ).astype(bf)
        invlen_rep = np.ascontiguousarray(
            np.broadcast_to((1.0 / lens).reshape(1, BL), (128, BL))
        )
        tok_idx = np.ascontiguousarray(tok.T.astype(np.int32))  # [L=128, BL]
        msel = sel.copy()
        for e in range(BL):
            msel[BL + e, e * L : (e + 1) * L] = -30.0 * (1.0 - mask[e])
        m = dict(shared)
        m.update(tok_idx=tok_idx, mask_rep=mask_rep, invlen_rep=invlen_rep,
                 msel=msel)
        in_maps.append(m)
    return in_maps


_NC_CACHE = {}


def kernel(**inputs) -> np.ndarray:
    in_maps = prep_in_maps(inputs)
    if "nc" not in _NC_CACHE:
        _NC_CACHE["nc"] = build_nc()
    nc = _NC_CACHE["nc"]
    res = run_bass_kernel_spmd(nc, in_maps, core_ids=list(range(NCORES)))
    return np.concatenate([r["out"] for r in res.results], axis=0)


if __name__ == "__main__":
    nc = build_nc()
    print("built ok")

